# revision 13
# baseline (speedup 1.0000x reference)
"""Trainium2 Bass kernel for AffineGPT2Attention (B=4, S=1024, D=1024, H=16).

Sharding: 8 cores = 4 batches x 2 sequence-shards. Core c handles batch
c//2 and query blocks {r, r+2} (r = c%2, blocks of 256 queries) --
causally balanced, no cross-core communication. K/V are computed for the
full sequence on both cores of a pair (duplicated work is cheaper than
collectives on this stack).

All matmul operands are float16 (full PE rate, fast weight load, fp32
PSUM accumulation; fp16 mantissa keeps end-to-end error ~1e-3).

Per-core dataflow:
  1. Q/K projected feature-major (QT/KT = [dh, s]); V token-major with a
     ones-column per head (V' = [V_h | 1]) so PV also yields softmax
     denominators.
  2. scoresT[sk, sq] tiles; q blocks j=0,1 processed jointly for sk
     tiles t<4 (N=512 matmuls), j=1 alone for t in 4..7. Scores land in
     2-bank PSUM tiles so one wide EXP drains 2 sk-tiles at once.
  3. Causal masking: multiplicative 0/1 fp16 masks (host data).
  4. PV accumulates [65, 512]; row 64 = denominators. reciprocal on DVE,
     K=1 matmul broadcasts it across partitions, normalize on DVE.
  5. c_proj fp16 (weights pre-scaled by affine_w; all biases folded into
     one per-feature output bias), bias on ACT, DMA out fp32.
"""

from contextlib import ExitStack

import numpy as np

import concourse.bass as bass
import concourse.mybir as mybir
import concourse.tile as tile
from concourse import bacc
from concourse.bass_utils import run_bass_kernel_spmd

B, S, D, H, Dh = 4, 1024, 1024, 16, 64
P = 128
NKT = D // P          # 8 contraction tiles
NFT = 8               # feature tiles per Q / K (1024/128)
NST = S // P          # 8 sequence tiles
BLK = 256             # query block width
VW = Dh + 1           # V' per-head width (64 + ones column)
N_CORES = 8

F32 = mybir.dt.float32
F16 = mybir.dt.float16
Id = mybir.ActivationFunctionType.Identity
Exp = mybir.ActivationFunctionType.Exp


def build_nc():
    nc = bacc.Bacc("TRN2", target_bir_lowering=False, debug=False,
                   num_devices=N_CORES)
    t_xt = nc.dram_tensor("xt", [D, S], F16, kind="ExternalInput")
    t_xq = nc.dram_tensor("xq", [D, 2 * BLK], F16, kind="ExternalInput")
    t_wqkv = nc.dram_tensor("wqkv", [D, 3 * D], F16, kind="ExternalInput")
    t_wp = nc.dram_tensor("wp", [D, D], F16, kind="ExternalInput")
    t_bqk = nc.dram_tensor("bqk", [P, 16], F32, kind="ExternalInput")
    t_bp = nc.dram_tensor("bp", [P, NFT], F32, kind="ExternalInput")
    # mj[t]: [mask_r[t] | ones]  (joint j0|j1 mask for sk tile t<4)
    t_mj = nc.dram_tensor("mj", [4, P, 2 * BLK], F16, kind="ExternalInput")
    # m2[s]: mask_r[s] applied at sk tile t=4+s for j=1
    t_m2 = nc.dram_tensor("m2", [4, P, BLK], F16, kind="ExternalInput")
    t_out = nc.dram_tensor("out", [D, 2 * BLK], F32, kind="ExternalOutput")

    with tile.TileContext(nc) as tc:
        emit(nc, tc, t_xt, t_xq, t_wqkv, t_wp, t_bqk, t_bp, t_mj, t_m2, t_out)
    nc.finalize()
    return nc


def emit(nc, tc, t_xt, t_xq, t_wqkv, t_wp, t_bqk, t_bp, t_mj, t_m2, t_out):
    ctx = ExitStack()
    res = ctx.enter_context(tc.tile_pool(name="res", bufs=1))
    dram = ctx.enter_context(tc.tile_pool(name="dram", bufs=1, space="DRAM"))
    wpool = ctx.enter_context(tc.tile_pool(name="wpool", bufs=2))
    epool = ctx.enter_context(tc.tile_pool(name="epool", bufs=3))
    opool = ctx.enter_context(tc.tile_pool(name="opool", bufs=2))
    spool = ctx.enter_context(tc.tile_pool(name="spool", bufs=2))

    with ctx:
        # ---- resident SBUF tensors ----
        xt_sb = res.tile([P, NKT, S], F16, tag="xt")
        xq_sb = res.tile([P, NKT, 2 * BLK], F16, tag="xq")
        qt_sb = res.tile([P, NFT, 2 * BLK], F16, tag="qt")
        kt_sb = res.tile([P, NFT, S], F16, tag="kt")
        vp_sb = res.tile([P, NST, H * VW], F16, tag="vp")
        at_sb = res.tile([P, NKT, 2 * BLK], F16, tag="at")
        mj_sb = res.tile([P, 4, 2 * BLK], F16, tag="mj")
        m2_sb = res.tile([P, 4, BLK], F16, tag="m2")
        bqk_sb = res.tile([P, 16], F32, tag="bqk")
        bp_sb = res.tile([P, NFT], F32, tag="bp")
        ones_sb = res.tile([1, Dh], F16, tag="ones")
        at_un = res.tile([P, H, 2 * BLK], F32, tag="at_un")
        den_all = res.tile([P, H, 4], F32, tag="den_all")
        rec_all = res.tile([P, H, 4], F16, tag="rec_all")

        dden = dram.tile([H, 2 * BLK], F32, tag="dden")
        drec = dram.tile([H, 2 * BLK], F16, tag="drec")
        nc.vector.memset(ones_sb, 1.0)
        nc.sync.dma_start(bqk_sb, t_bqk.ap())
        nc.sync.dma_start(bp_sb, t_bp.ap())
        nc.sync.dma_start(mj_sb, t_mj.ap().transpose([1, 0, 2]))
        nc.sync.dma_start(m2_sb, t_m2.ap().transpose([1, 0, 2]))
        for kt in range(NKT):
            nc.sync.dma_start(xt_sb[:, kt, :], t_xt.ap()[kt * P:(kt + 1) * P, :])
        nc.sync.dma_start(xq_sb, t_xq.ap().rearrange("(k p) c -> p k c", p=P))
        # ones columns of V' (data columns are overwritten by the V drain)
        for st in range(NST):
            nc.vector.memset(
                vp_sb[:, st, :].rearrange("p (h w) -> p h w", w=VW)[:, :, Dh:],
                1.0)

        # ---- phase A: QKV projections ----
        with tc.tile_pool(name="ps_a", bufs=2, space="PSUM") as ps_a:
            for ft in range(16):        # 0-7: Q feature tiles, 8-15: K
                w_ft = wpool.tile([P, NKT, P], F16, tag="wft")
                c0 = P * ft
                nc.sync.dma_start(
                    w_ft,
                    t_wqkv.ap()[:, c0:c0 + P].rearrange("(k p) c -> p k c", p=P))
                if ft < 8:              # Q: core's 512 selected queries
                    ps_q = ps_a.tile([P, 2 * BLK], F32, tag="qk")
                    for kt in range(NKT):
                        nc.tensor.matmul(
                            ps_q, w_ft[:, kt, :], xq_sb[:, kt, :],
                            start=(kt == 0), stop=(kt == NKT - 1))
                    nc.scalar.activation(qt_sb[:, ft, :], ps_q, Id,
                                         bias=bqk_sb[:, ft:ft + 1], scale=1.0)
                else:                   # K: full sequence, two 512 chunks
                    for sc in range(2):
                        ps_k = ps_a.tile([P, 512], F32, tag="qk")
                        for kt in range(NKT):
                            nc.tensor.matmul(
                                ps_k, w_ft[:, kt, :],
                                xt_sb[:, kt, 512 * sc:512 * (sc + 1)],
                                start=(kt == 0), stop=(kt == NKT - 1))
                        nc.scalar.activation(
                            kt_sb[:, ft - 8, 512 * sc:512 * (sc + 1)], ps_k,
                            Id, bias=bqk_sb[:, ft:ft + 1], scale=1.0)

            # V projection (token-major), 2 col-halves x 4 st-groups
            for half in range(2):
                wv_h = wpool.tile([P, NKT, 512], F16, tag="wvh", bufs=1)
                nc.sync.dma_start(
                    wv_h,
                    t_wqkv.ap()[:, 2 * D + 512 * half: 2 * D + 512 * (half + 1)]
                    .rearrange("(k p) c -> p k c", p=P))
                for sg in range(4):
                    ps_vs = [ps_a.tile([P, 512], F32, tag="qk", name=f"psv{si}")
                             for si in range(2)]
                    for kt in range(NKT):
                        for si in range(2):
                            st = 2 * sg + si
                            nc.tensor.matmul(
                                ps_vs[si], xt_sb[:, kt, st * P:(st + 1) * P],
                                wv_h[:, kt, :],
                                start=(kt == 0), stop=(kt == NKT - 1))
                    for si in range(2):
                        st = 2 * sg + si
                        dst = (vp_sb[:, st, :]
                               .rearrange("p (h w) -> p h w", w=VW)
                               [:, 8 * half:8 * (half + 1), :Dh])
                        nc.scalar.activation(
                            dst, ps_vs[si].rearrange("p (h w) -> p h w", w=Dh),
                            Id, bias=0.0, scale=1.0)

        # ---- phase B: attention ----
        with (tc.tile_pool(name="ps_s", bufs=2, space="PSUM") as ps_s,
              tc.tile_pool(name="ps_o", bufs=4, space="PSUM") as ps_o):
            for h in range(H):
                ft, row = h // 2, Dh * (h % 2)
                kh = kt_sb[row:row + Dh, ft, :]
                qh = qt_sb[row:row + Dh, ft, :]
                ps_pv = ps_o.tile([P, 2 * BLK], F32, tag="pv")
                # sk tiles 0..3: joint over both query blocks (N=512)
                for tp in range(2):     # t-pairs (0,1), (2,3)
                    ps_sc = ps_s.tile([P, 4 * BLK], F32, tag="s")
                    for ti in range(2):
                        t = 2 * tp + ti
                        nc.tensor.matmul(
                            ps_sc[:, ti * 512:(ti + 1) * 512],
                            kh[:, t * P:(t + 1) * P], qh,
                            start=True, stop=True)
                    e_t = epool.tile([P, 4 * BLK], F16, tag="e")
                    nc.scalar.activation(e_t, ps_sc, Exp, bias=0.0, scale=0.125)
                    nc.vector.tensor_mul(
                        e_t, e_t,
                        mj_sb[:, 2 * tp:2 * tp + 2, :]
                        .rearrange("p a b -> p (a b)"))
                    for ti in range(2):
                        t = 2 * tp + ti
                        nc.tensor.matmul(
                            ps_pv[:VW, :], vp_sb[:, t, VW * h:VW * (h + 1)],
                            e_t[:, ti * 512:(ti + 1) * 512],
                            start=(t == 0), stop=False)
                # sk tiles 4..7: j=1 only (N=256)
                ps_sc2 = ps_s.tile([P, 4 * BLK], F32, tag="s")
                for t in range(4, 8):
                    nc.tensor.matmul(
                        ps_sc2[:, (t - 4) * BLK:(t - 3) * BLK],
                        kh[:, t * P:(t + 1) * P], qh[:, BLK:],
                        start=True, stop=True)
                e2 = epool.tile([P, 4 * BLK], F16, tag="e")
                nc.scalar.activation(e2, ps_sc2, Exp, bias=0.0, scale=0.125)
                nc.vector.tensor_mul(
                    e2, e2, m2_sb.rearrange("p a b -> p (a b)"))
                for t in range(4, 8):
                    nc.tensor.matmul(
                        ps_pv[:VW, BLK:], vp_sb[:, t, VW * h:VW * (h + 1)],
                        e2[:, (t - 4) * BLK:(t - 3) * BLK],
                        start=False, stop=(t == 7))
                # drain PV (with denominator row) to SBUF, freeing PSUM
                nc.vector.tensor_copy(at_un[:VW, h, :], ps_pv[:VW, :])
                # reshape the [1,512] denom row to [128,4] via a DRAM
                # round-trip so the DVE reciprocal runs at free-size 4
                nc.sync.dma_start(dden[h, :], at_un[Dh:VW, h, :])
                nc.sync.dma_start(
                    den_all[:, h, :],
                    dden[h, :].rearrange("(f p) -> p f", p=P))
                with nc.allow_low_precision(reason="fp16 softmax denom"):
                    nc.vector.reciprocal(rec_all[:, h, :], den_all[:, h, :])
                nc.sync.dma_start(
                    drec[h, :].rearrange("(f p) -> p f", p=P),
                    rec_all[:, h, :])
                recip = spool.tile([1, 2 * BLK], F16, tag="recip")
                nc.sync.dma_start(recip, drec[h:h + 1, :])
                ps_bc = ps_o.tile([Dh, 2 * BLK], F32, tag="pv", name="ps_bc")
                nc.tensor.matmul(ps_bc, ones_sb, recip, start=True, stop=True)
                nc.vector.scalar_tensor_tensor(
                    at_sb[row:row + Dh, ft, :], ps_bc, 1.0, at_un[:Dh, h, :],
                    op0=mybir.AluOpType.mult, op1=mybir.AluOpType.mult)

        # ---- phase C: c_proj + bias, DMA out ----
        with tc.tile_pool(name="ps_p", bufs=2, space="PSUM") as ps_p:
            for nt in range(NFT):
                w_nt = wpool.tile([P, NKT, P], F16, tag="wft", name="w_nt")
                nc.sync.dma_start(
                    w_nt, t_wp.ap()[:, nt * P:(nt + 1) * P]
                    .rearrange("(k p) c -> p k c", p=P))
                ps_pr = ps_p.tile([P, 2 * BLK], F32, tag="proj")
                for kt in range(NKT):
                    nc.tensor.matmul(ps_pr, w_nt[:, kt, :], at_sb[:, kt, :],
                                     start=(kt == 0), stop=(kt == NKT - 1))
                o_t = opool.tile([P, 2 * BLK], F32, tag="o")
                nc.scalar.activation(o_t, ps_pr, Id, bias=bp_sb[:, nt:nt + 1],
                                     scale=1.0)
                nc.sync.dma_start(t_out.ap()[nt * P:(nt + 1) * P, :], o_t)


_NC_CACHE = None


def _get_nc():
    global _NC_CACHE
    if _NC_CACHE is None:
        _NC_CACHE = build_nc()
    return _NC_CACHE


def make_in_maps(hidden_states, c_attn_w, c_attn_b, c_proj_w, c_proj_b,
                 affine_w, affine_b):
    hidden_states = np.asarray(hidden_states, dtype=np.float32)
    c_attn_w = np.asarray(c_attn_w, dtype=np.float32)
    c_attn_b = np.asarray(c_attn_b, dtype=np.float32)
    c_proj_w = np.asarray(c_proj_w, dtype=np.float32)
    c_proj_b = np.asarray(c_proj_b, dtype=np.float32)
    affine_w = np.asarray(affine_w, dtype=np.float32)
    affine_b = np.asarray(affine_b, dtype=np.float32)

    wqkv = c_attn_w.astype(np.float16)
    wp = (c_proj_w * affine_w[None, :]).astype(np.float16)
    # all biases folded into a per-output-feature bias (v-bias rides
    # through softmax rows summing to 1, then through c_proj)
    bv = c_attn_b[2 * D:]
    bp_full = (bv @ c_proj_w + c_proj_b) * affine_w + affine_b
    bp = np.ascontiguousarray(bp_full.reshape(NFT, P).T)
    bqk = np.ascontiguousarray(c_attn_b[:2 * D].reshape(16, P).T)

    ii, mm = np.arange(P)[:, None], np.arange(BLK)[None, :]
    dA = (mm >= ii).astype(np.float16)
    dB = (mm >= ii + P).astype(np.float16)
    ones = np.ones((P, BLK), np.float16)
    zeros = np.zeros((P, BLK), np.float16)
    masks_r = [np.stack([dA, dB, zeros, zeros]),
               np.stack([ones, ones, dA, dB])]

    in_maps = []
    for c in range(N_CORES):
        b, r = c // 2, c % 2
        xt32 = hidden_states[b].T
        xt = np.ascontiguousarray(xt32).astype(np.float16)
        sel = np.r_[BLK * r:BLK * (r + 1), BLK * (r + 2):BLK * (r + 3)]
        xq = np.ascontiguousarray(xt32[:, sel]).astype(np.float16)
        mj = np.concatenate(
            [masks_r[r], np.broadcast_to(ones, (4, P, BLK))], axis=2)
        in_maps.append({
            "xt": xt, "xq": xq, "wqkv": wqkv, "wp": wp,
            "bqk": bqk, "bp": bp,
            "mj": np.ascontiguousarray(mj), "m2": masks_r[r],
        })
    return in_maps


def assemble(results):
    out = np.empty((B, S, D), np.float32)
    for c in range(N_CORES):
        b, r = c // 2, c % 2
        oc = results[c]["out"]          # [D, 512]
        for j in range(2):
            bj = r + 2 * j
            out[b, BLK * bj:BLK * (bj + 1), :] = oc[:, BLK * j:BLK * (j + 1)].T
    return out


def kernel(hidden_states, c_attn_w, c_attn_b, c_proj_w, c_proj_b,
           affine_w, affine_b, _trace=False):
    nc = _get_nc()
    in_maps = make_in_maps(hidden_states, c_attn_w, c_attn_b, c_proj_w,
                           c_proj_b, affine_w, affine_b)
    res = run_bass_kernel_spmd(nc, in_maps, core_ids=list(range(N_CORES)),
                               trace=_trace)
    out = assemble(res.results)
    if _trace:
        kernel.last_exec_time_ns = res.exec_time_ns
        kernel.last_results = res
    return out


# revision 15
# speedup vs baseline: 2.6239x; 2.6239x over previous
"""Trainium2 Bass kernel for AffineGPT2Attention (B=4, S=1024, D=1024, H=16).

Sharding: 8 cores = 4 batches x 2 sequence-shards. Core c handles batch
c//2 and query blocks {r, r+2} (r = c%2, blocks of 256 queries) --
causally balanced, no cross-core communication. K/V are computed for the
full sequence on both cores of a pair (duplicated work is cheaper than
collectives on this stack).

All matmul operands are float16 (full PE rate, fast weight load, fp32
PSUM accumulation; fp16 mantissa keeps end-to-end error ~1e-3).

Per-core dataflow:
  1. Q/K projected feature-major (QT/KT = [dh, s]); V token-major with a
     ones-column per head (V' = [V_h | 1]) so PV also yields softmax
     denominators.
  2. scoresT[sk, sq] tiles; q blocks j=0,1 processed jointly for sk
     tiles t<4 (N=512 matmuls), j=1 alone for t in 4..7. Scores land in
     2-bank PSUM tiles so one wide EXP drains 2 sk-tiles at once.
  3. Causal masking: multiplicative 0/1 fp16 masks (host data).
  4. PV accumulates [65, 512]; row 64 = denominators. reciprocal on DVE,
     K=1 matmul broadcasts it across partitions, normalize on DVE.
  5. c_proj fp16 (weights pre-scaled by affine_w; all biases folded into
     one per-feature output bias), bias on ACT, DMA out fp32.
"""

from contextlib import ExitStack

import numpy as np

import concourse.bass as bass
import concourse.mybir as mybir
import concourse.tile as tile
from concourse import bacc
from concourse.bass_utils import run_bass_kernel_spmd

B, S, D, H, Dh = 4, 1024, 1024, 16, 64
P = 128
NKT = D // P          # 8 contraction tiles
NFT = 8               # feature tiles per Q / K (1024/128)
NST = S // P          # 8 sequence tiles
BLK = 256             # query block width
VW = Dh + 1           # V' per-head width (64 + ones column)
N_CORES = 8

F32 = mybir.dt.float32
F16 = mybir.dt.float16
Id = mybir.ActivationFunctionType.Identity
Exp = mybir.ActivationFunctionType.Exp


def build_nc():
    nc = bacc.Bacc("TRN2", target_bir_lowering=False, debug=False,
                   num_devices=N_CORES)
    t_xt = nc.dram_tensor("xt", [D, S], F16, kind="ExternalInput")
    t_xq = nc.dram_tensor("xq", [D, 2 * BLK], F16, kind="ExternalInput")
    t_wqkv = nc.dram_tensor("wqkv", [D, 3 * D], F16, kind="ExternalInput")
    t_wp = nc.dram_tensor("wp", [D, D], F16, kind="ExternalInput")
    t_bqk = nc.dram_tensor("bqk", [P, 16], F32, kind="ExternalInput")
    t_bp = nc.dram_tensor("bp", [P, NFT], F32, kind="ExternalInput")
    # mj[t]: [mask_r[t] | ones]  (joint j0|j1 mask for sk tile t<4)
    t_mj = nc.dram_tensor("mj", [4, P, 2 * BLK], F16, kind="ExternalInput")
    # m2[s]: mask_r[s] applied at sk tile t=4+s for j=1
    t_m2 = nc.dram_tensor("m2", [4, P, BLK], F16, kind="ExternalInput")
    t_out = nc.dram_tensor("out", [D, 2 * BLK], F32, kind="ExternalOutput")

    with tile.TileContext(nc) as tc:
        emit(nc, tc, t_xt, t_xq, t_wqkv, t_wp, t_bqk, t_bp, t_mj, t_m2, t_out)
    nc.finalize()
    return nc


def emit(nc, tc, t_xt, t_xq, t_wqkv, t_wp, t_bqk, t_bp, t_mj, t_m2, t_out):
    ctx = ExitStack()
    res = ctx.enter_context(tc.tile_pool(name="res", bufs=1))
    wpool = ctx.enter_context(tc.tile_pool(name="wpool", bufs=2))
    epool = ctx.enter_context(tc.tile_pool(name="epool", bufs=3))
    opool = ctx.enter_context(tc.tile_pool(name="opool", bufs=2))
    spool = ctx.enter_context(tc.tile_pool(name="spool", bufs=2))

    with ctx:
        # ---- resident SBUF tensors ----
        xt_sb = res.tile([P, NKT, S], F16, tag="xt")
        xq_sb = res.tile([P, NKT, 2 * BLK], F16, tag="xq")
        qt_sb = res.tile([P, NFT, 2 * BLK], F16, tag="qt")
        kt_sb = res.tile([P, NFT, S], F16, tag="kt")
        vp_sb = res.tile([P, NST, H * VW], F16, tag="vp")
        at_sb = res.tile([P, NKT, 2 * BLK], F16, tag="at")
        mj_sb = res.tile([P, 4, 2 * BLK], F16, tag="mj")
        m2_sb = res.tile([P, 4, BLK], F16, tag="m2")
        bqk_sb = res.tile([P, 16], F32, tag="bqk")
        bp_sb = res.tile([P, NFT], F32, tag="bp")
        ones_sb = res.tile([1, Dh], F16, tag="ones")
        at_un = res.tile([P, H, 2 * BLK], F32, tag="at_un")

        nc.vector.memset(ones_sb, 1.0)
        nc.sync.dma_start(bqk_sb, t_bqk.ap())
        nc.sync.dma_start(bp_sb, t_bp.ap())
        nc.sync.dma_start(mj_sb, t_mj.ap().transpose([1, 0, 2]))
        nc.sync.dma_start(m2_sb, t_m2.ap().transpose([1, 0, 2]))
        for kt in range(NKT):
            nc.sync.dma_start(xt_sb[:, kt, :], t_xt.ap()[kt * P:(kt + 1) * P, :])
        nc.sync.dma_start(xq_sb, t_xq.ap().rearrange("(k p) c -> p k c", p=P))
        # ones columns of V' (data columns are overwritten by the V drain)
        for st in range(NST):
            nc.vector.memset(
                vp_sb[:, st, :].rearrange("p (h w) -> p h w", w=VW)[:, :, Dh:],
                1.0)

        # ---- phase A: QKV projections ----
        with tc.tile_pool(name="ps_a", bufs=2, space="PSUM") as ps_a:
            for ft in range(16):        # 0-7: Q feature tiles, 8-15: K
                w_ft = wpool.tile([P, NKT, P], F16, tag="wft")
                c0 = P * ft
                nc.sync.dma_start(
                    w_ft,
                    t_wqkv.ap()[:, c0:c0 + P].rearrange("(k p) c -> p k c", p=P))
                if ft < 8:              # Q: core's 512 selected queries
                    ps_q = ps_a.tile([P, 2 * BLK], F32, tag="qk")
                    for kt in range(NKT):
                        nc.tensor.matmul(
                            ps_q, w_ft[:, kt, :], xq_sb[:, kt, :],
                            start=(kt == 0), stop=(kt == NKT - 1))
                    nc.scalar.activation(qt_sb[:, ft, :], ps_q, Id,
                                         bias=bqk_sb[:, ft:ft + 1], scale=1.0)
                else:                   # K: full sequence, two 512 chunks
                    for sc in range(2):
                        ps_k = ps_a.tile([P, 512], F32, tag="qk")
                        for kt in range(NKT):
                            nc.tensor.matmul(
                                ps_k, w_ft[:, kt, :],
                                xt_sb[:, kt, 512 * sc:512 * (sc + 1)],
                                start=(kt == 0), stop=(kt == NKT - 1))
                        nc.scalar.activation(
                            kt_sb[:, ft - 8, 512 * sc:512 * (sc + 1)], ps_k,
                            Id, bias=bqk_sb[:, ft:ft + 1], scale=1.0)

            # V projection (token-major), 2 col-halves x 4 st-groups
            for half in range(2):
                wv_h = wpool.tile([P, NKT, 512], F16, tag="wvh", bufs=1)
                nc.sync.dma_start(
                    wv_h,
                    t_wqkv.ap()[:, 2 * D + 512 * half: 2 * D + 512 * (half + 1)]
                    .rearrange("(k p) c -> p k c", p=P))
                for sg in range(4):
                    ps_vs = [ps_a.tile([P, 512], F32, tag="qk", name=f"psv{si}")
                             for si in range(2)]
                    for kt in range(NKT):
                        for si in range(2):
                            st = 2 * sg + si
                            nc.tensor.matmul(
                                ps_vs[si], xt_sb[:, kt, st * P:(st + 1) * P],
                                wv_h[:, kt, :],
                                start=(kt == 0), stop=(kt == NKT - 1))
                    for si in range(2):
                        st = 2 * sg + si
                        dst = (vp_sb[:, st, :]
                               .rearrange("p (h w) -> p h w", w=VW)
                               [:, 8 * half:8 * (half + 1), :Dh])
                        nc.scalar.activation(
                            dst, ps_vs[si].rearrange("p (h w) -> p h w", w=Dh),
                            Id, bias=0.0, scale=1.0)

        # ---- phase B: attention ----
        with (tc.tile_pool(name="ps_s", bufs=2, space="PSUM") as ps_s,
              tc.tile_pool(name="ps_o", bufs=4, space="PSUM") as ps_o):
            for h in range(H):
                ft, row = h // 2, Dh * (h % 2)
                kh = kt_sb[row:row + Dh, ft, :]
                qh = qt_sb[row:row + Dh, ft, :]
                ps_pv = ps_o.tile([P, 2 * BLK], F32, tag="pv")
                # sk tiles 0..3: joint over both query blocks (N=512)
                for tp in range(2):     # t-pairs (0,1), (2,3)
                    ps_sc = ps_s.tile([P, 4 * BLK], F32, tag="s")
                    for ti in range(2):
                        t = 2 * tp + ti
                        nc.tensor.matmul(
                            ps_sc[:, ti * 512:(ti + 1) * 512],
                            kh[:, t * P:(t + 1) * P], qh,
                            start=True, stop=True)
                    e_t = epool.tile([P, 4 * BLK], F16, tag="e")
                    nc.scalar.activation(e_t, ps_sc, Exp, bias=0.0, scale=0.125)
                    nc.vector.tensor_mul(
                        e_t, e_t,
                        mj_sb[:, 2 * tp:2 * tp + 2, :]
                        .rearrange("p a b -> p (a b)"))
                    for ti in range(2):
                        t = 2 * tp + ti
                        nc.tensor.matmul(
                            ps_pv[:VW, :], vp_sb[:, t, VW * h:VW * (h + 1)],
                            e_t[:, ti * 512:(ti + 1) * 512],
                            start=(t == 0), stop=False)
                # sk tiles 4..7: j=1 only (N=256)
                ps_sc2 = ps_s.tile([P, 4 * BLK], F32, tag="s")
                for t in range(4, 8):
                    nc.tensor.matmul(
                        ps_sc2[:, (t - 4) * BLK:(t - 3) * BLK],
                        kh[:, t * P:(t + 1) * P], qh[:, BLK:],
                        start=True, stop=True)
                e2 = epool.tile([P, 4 * BLK], F16, tag="e")
                nc.scalar.activation(e2, ps_sc2, Exp, bias=0.0, scale=0.125)
                nc.vector.tensor_mul(
                    e2, e2, m2_sb.rearrange("p a b -> p (a b)"))
                for t in range(4, 8):
                    nc.tensor.matmul(
                        ps_pv[:VW, BLK:], vp_sb[:, t, VW * h:VW * (h + 1)],
                        e2[:, (t - 4) * BLK:(t - 3) * BLK],
                        start=False, stop=(t == 7))
                # drain PV to SBUF, freeing PSUM; denominator row goes to a
                # partition-0 tile (reciprocal_approx_fast requires base 0)
                nc.vector.tensor_copy(at_un[:Dh, h, :], ps_pv[:Dh, :])
                den_row = spool.tile([1, 2 * BLK], F32, tag="den_row")
                nc.vector.tensor_copy(den_row, ps_pv[Dh:VW, :])
                # fast approximate reciprocal of the denominator row, then
                # partition-broadcast it on the (otherwise idle) GPSIMD
                recip = spool.tile([1, 2 * BLK], F32, tag="recip")
                nc.vector.reciprocal_approx_fast(recip, den_row)
                bc_sb = spool.tile([Dh, 2 * BLK], F32, tag="bc")
                nc.gpsimd.partition_broadcast(bc_sb, recip, channels=Dh)
                nc.vector.scalar_tensor_tensor(
                    at_sb[row:row + Dh, ft, :], bc_sb, 1.0, at_un[:Dh, h, :],
                    op0=mybir.AluOpType.mult, op1=mybir.AluOpType.mult)

        # ---- phase C: c_proj + bias, DMA out ----
        with tc.tile_pool(name="ps_p", bufs=2, space="PSUM") as ps_p:
            for nt in range(NFT):
                w_nt = wpool.tile([P, NKT, P], F16, tag="wft", name="w_nt")
                nc.sync.dma_start(
                    w_nt, t_wp.ap()[:, nt * P:(nt + 1) * P]
                    .rearrange("(k p) c -> p k c", p=P))
                ps_pr = ps_p.tile([P, 2 * BLK], F32, tag="proj")
                for kt in range(NKT):
                    nc.tensor.matmul(ps_pr, w_nt[:, kt, :], at_sb[:, kt, :],
                                     start=(kt == 0), stop=(kt == NKT - 1))
                o_t = opool.tile([P, 2 * BLK], F32, tag="o")
                nc.scalar.activation(o_t, ps_pr, Id, bias=bp_sb[:, nt:nt + 1],
                                     scale=1.0)
                nc.sync.dma_start(t_out.ap()[nt * P:(nt + 1) * P, :], o_t)


_NC_CACHE = None


def _get_nc():
    global _NC_CACHE
    if _NC_CACHE is None:
        _NC_CACHE = build_nc()
    return _NC_CACHE


def make_in_maps(hidden_states, c_attn_w, c_attn_b, c_proj_w, c_proj_b,
                 affine_w, affine_b):
    hidden_states = np.asarray(hidden_states, dtype=np.float32)
    c_attn_w = np.asarray(c_attn_w, dtype=np.float32)
    c_attn_b = np.asarray(c_attn_b, dtype=np.float32)
    c_proj_w = np.asarray(c_proj_w, dtype=np.float32)
    c_proj_b = np.asarray(c_proj_b, dtype=np.float32)
    affine_w = np.asarray(affine_w, dtype=np.float32)
    affine_b = np.asarray(affine_b, dtype=np.float32)

    wqkv = c_attn_w.astype(np.float16)
    wp = (c_proj_w * affine_w[None, :]).astype(np.float16)
    # all biases folded into a per-output-feature bias (v-bias rides
    # through softmax rows summing to 1, then through c_proj)
    bv = c_attn_b[2 * D:]
    bp_full = (bv @ c_proj_w + c_proj_b) * affine_w + affine_b
    bp = np.ascontiguousarray(bp_full.reshape(NFT, P).T)
    bqk = np.ascontiguousarray(c_attn_b[:2 * D].reshape(16, P).T)

    ii, mm = np.arange(P)[:, None], np.arange(BLK)[None, :]
    dA = (mm >= ii).astype(np.float16)
    dB = (mm >= ii + P).astype(np.float16)
    ones = np.ones((P, BLK), np.float16)
    zeros = np.zeros((P, BLK), np.float16)
    masks_r = [np.stack([dA, dB, zeros, zeros]),
               np.stack([ones, ones, dA, dB])]

    in_maps = []
    for c in range(N_CORES):
        b, r = c // 2, c % 2
        xt32 = hidden_states[b].T
        xt = np.ascontiguousarray(xt32).astype(np.float16)
        sel = np.r_[BLK * r:BLK * (r + 1), BLK * (r + 2):BLK * (r + 3)]
        xq = np.ascontiguousarray(xt32[:, sel]).astype(np.float16)
        mj = np.concatenate(
            [masks_r[r], np.broadcast_to(ones, (4, P, BLK))], axis=2)
        in_maps.append({
            "xt": xt, "xq": xq, "wqkv": wqkv, "wp": wp,
            "bqk": bqk, "bp": bp,
            "mj": np.ascontiguousarray(mj), "m2": masks_r[r],
        })
    return in_maps


def assemble(results):
    out = np.empty((B, S, D), np.float32)
    for c in range(N_CORES):
        b, r = c // 2, c % 2
        oc = results[c]["out"]          # [D, 512]
        for j in range(2):
            bj = r + 2 * j
            out[b, BLK * bj:BLK * (bj + 1), :] = oc[:, BLK * j:BLK * (j + 1)].T
    return out


def kernel(hidden_states, c_attn_w, c_attn_b, c_proj_w, c_proj_b,
           affine_w, affine_b, _trace=False):
    nc = _get_nc()
    in_maps = make_in_maps(hidden_states, c_attn_w, c_attn_b, c_proj_w,
                           c_proj_b, affine_w, affine_b)
    res = run_bass_kernel_spmd(nc, in_maps, core_ids=list(range(N_CORES)),
                               trace=_trace)
    out = assemble(res.results)
    if _trace:
        kernel.last_exec_time_ns = res.exec_time_ns
        kernel.last_results = res
    return out
